# revision 18
# baseline (speedup 1.0000x reference)
"""LJ potential + two-level segment sum (edges -> atoms -> molecules) on 8 trn2 cores.

Strategy ("shard edges by molecule"):
  The final output is only per-molecule [1000]. On the host (as the sharding
  step) we compute each edge's molecule id m_e = idx_m[idx_i[e]], order edges
  by molecule, and pad each molecule's edge run to a multiple of 512 so every
  512-edge group (= one SBUF partition row per tile) is molecule-pure. Pad
  edges use vec=(1e3,0,0): r=1000 > cutoff => switch==0 => exactly zero energy.

  Each core gets a contiguous slice of the padded stream as [T, 128, 3*F]
  tiles (x/y/z planes, F=512; partition p of tile t holds edges
  [(t*128+p)*512, +512) of the core's slice). The device computes per-edge
  LJ energy with fused custom DVE ops + ACT + one GPSIMD add; the final
  fused DVE op carries accum=ADD, so its accum_out [128,1] directly yields
  the per-group (512-edge) sums. The [128, T] group sums are DMA'd back;
  the host slices them at (host-known) molecule group boundaries.

Per-edge math (matches reference exactly up to fp rounding):
  r2 = x^2+y^2+z^2 ;  u = 1/r2 ;  p6 = u^3 ;  q = p6^2 - p6
  a  = relu(2r - 4), r = sqrt(r2) ;  b = relu(1 - a)   # == clamp(5-2r, 0, 1)
  switch = b^2*(3-2b)                                  # == reference switch
  y  = q * switch ;  out_m = 0.5 * sum_{e in m} y_e    (0.5 applied on host)
"""

import sys

import numpy as np

if "/opt/trn_rl_repo" not in sys.path:
    sys.path.insert(0, "/opt/trn_rl_repo")

N_EDGES = 20_000_000
N_ATOMS = 1_000_000
N_MOL = 1000
N_CORES = 8
P = 128          # SBUF partitions
F = 1024         # edges per group; molecule padding grain
GPT = 1          # groups per partition row per tile
FH = F * GPT     # tile free width (edges per partition row per tile)
PAD_X = 1.0e3    # pad edge vec = (1e3, 0, 0) -> r2 = 1e6 -> switch = 0 -> y = 0

_registered_ops = {}
_compiled_cache = {}


# ---------------------------------------------------------------- custom DVE ops
def _register_custom_ops():
    """Register the fused LJ ops in concourse's custom-DVE table (idempotent)."""
    if _registered_ops:
        return _registered_ops

    from concourse import dve_ops as D
    from concourse.dve_spec import (
        AluOp, Spec, Src0, Src1, C0, One, lower, relu, sq, _has_src1,
    )
    from concourse.dve_uop import DveOpSpec

    def _shas(spec):
        out = {}
        for ver in ("v3", "v4"):
            s = DveOpSpec(
                name="tmp", opcode=1, uops=lower(spec, ver=ver), rd1_en=_has_src1(spec)
            )
            out[ver] = s.sha(ver)
        return out

    def _add(name, spec):
        existing = {op.name: op for op in D.OPS}
        if name in existing:
            _registered_ops[name] = existing[name]
            return
        op = D.DveOp(name, spec, subdim=False, uops_sha=_shas(spec))
        D.OPS.append(op)
        D._SUB_OPCODE_FOR_NAME[name] = D._CUSTOM_DVE_ROW_BASE + len(D.OPS) - 1
        assert D._SUB_OPCODE_FOR_NAME[name] < 0x20
        D.CUSTOM_DVE_SPECS[name] = spec
        _registered_ops[name] = op

    _add(
        "LJ_SQSUM2",
        Spec(
            body=sq(Src0) + sq(Src1),
            reference=lambda in0, in1, s0, s1, imm2: in0 * in0 + in1 * in1,
        ),
    )
    u3 = Src0 * Src0 * Src0
    _add(
        "LJ_Q",
        Spec(
            body=sq(u3) - u3,
            reference=lambda in0, in1, s0, s1, imm2: (
                (in0.astype(np.float32) ** 3) ** 2 - in0.astype(np.float32) ** 3
            ),
        ),
    )

    # in0=q, in1=a (=relu(2r-4)): b = relu(1-a); y = q * b^2 * (3-2b)
    # accum_out[p] = sum over the free dim of y  (per-group energy sum)
    def _ref_final2a_acc(in0, in1, s0, s1, imm2):
        b = np.maximum(1.0 - in1, 0.0).astype(np.float32)
        out = in0 * (b * b * (s0 - 2.0 * b))
        return out, out.sum(axis=-1, keepdims=True, dtype=np.float32)

    b_node = relu(One - Src1)
    _add(
        "LJ_FINAL2A_ACC",
        Spec(
            body=Src0 * (sq(b_node) * (C0 - (b_node + b_node))),
            accum=AluOp.ADD,
            reference=_ref_final2a_acc,
        ),
    )
    return _registered_ops


# ---------------------------------------------------------------- device kernel
def _build_kernel(T):
    """Build + compile the SPMD Bass program for T tiles per core."""
    if T in _compiled_cache:
        return _compiled_cache[T]

    import concourse.bacc as bacc
    import concourse.bass as bass
    import concourse.mybir as mybir
    import concourse.tile as tile
    from concourse.dve_ops import RECIPROCAL_APPROX_FAST

    ops = _register_custom_ops()
    f32 = mybir.dt.float32
    AF = mybir.ActivationFunctionType

    nc = bacc.Bacc("TRN2", target_bir_lowering=False, debug=False)
    v_dram = nc.dram_tensor("v", [T, P, 3 * FH], f32, kind="ExternalInput")
    out_dram = nc.dram_tensor("gsum", [P, T * GPT], f32, kind="ExternalOutput")

    with tile.TileContext(nc) as tc:
        with (
            tc.tile_pool(name="vin", bufs=4) as vin_pool,
            tc.tile_pool(name="work", bufs=3) as work,
            tc.tile_pool(name="cst", bufs=1) as cst,
        ):
            bias_m4 = cst.tile([P, 1], f32)
            nc.vector.memset(bias_m4[:], -4.0)
            gsum = cst.tile([P, T * GPT], f32)

            vt, x2t, yz2t, r2t = {}, {}, {}, {}

            # Software-pipelined emission: DMA 2 tiles ahead of the r2 stage,
            # which runs 2 tiles ahead of the energy/accumulate stage, so every
            # engine's program order only reaches ops whose inputs are ready.
            for ti in range(T + 4):
                if 0 <= ti - 4:
                    t = ti - 4
                    r2 = r2t.pop(t)
                    u = work.tile([P, FH], f32, tag="u")
                    nc.vector._custom_dve(RECIPROCAL_APPROX_FAST, out=u[:],
                                          in0=r2[:],
                                          s0=-0.23549792, s1=2.0017324, imm2=2.0)
                    r = work.tile([P, FH], f32, tag="r")
                    nc.scalar.activation(r[:], r2[:], AF.Sqrt)
                    a = work.tile([P, FH], f32, tag="a")
                    nc.scalar.activation(a[:], r[:], AF.Relu, bias=bias_m4[:],
                                         scale=2.0)
                    q = work.tile([P, FH], f32, tag="q")
                    nc.vector._custom_dve(ops["LJ_Q"], out=q[:], in0=u[:])

                    yv = work.tile([P, FH], f32, tag="yv")
                    nc.vector._custom_dve(
                        ops["LJ_FINAL2A_ACC"], out=yv[:], in0=q[:], in1=a[:],
                        s0=3.0, accum_out=gsum[:, t : t + 1],
                    )
                if 0 <= ti - 2 < T:
                    t = ti - 2
                    v = vt.pop(t)
                    x2 = work.tile([P, FH], f32, tag="x2")
                    nc.scalar.activation(x2[:], v[:, 0:FH], AF.Square)
                    yz2 = work.tile([P, FH], f32, tag="yz2")
                    nc.vector._custom_dve(
                        ops["LJ_SQSUM2"], out=yz2[:],
                        in0=v[:, FH : 2 * FH], in1=v[:, 2 * FH : 3 * FH],
                    )
                    r2 = work.tile([P, FH], f32, tag="r2", bufs=4)
                    nc.gpsimd.tensor_add(r2[:], x2[:], yz2[:])
                    r2t[t] = r2
                if ti < T:
                    t = ti
                    v = vin_pool.tile([P, 3 * FH], f32, tag="v")
                    nc.sync.dma_start(v[:], v_dram.ap()[t])
                    vt[t] = v

            nc.sync.dma_start(out_dram.ap()[:], gsum[:])

    nc.compile()
    _compiled_cache[T] = nc
    return nc


# ---------------------------------------------------------------- host prep
def _prepare(vec_ij, idx_i, idx_m):
    """Sort+pad edges by molecule; build per-core tile arrays and metadata.

    Returns (in_maps, T, gstart) where gstart[m] is molecule m's first
    512-edge group index in the global (concatenated-core) group order.
    """
    vec = np.ascontiguousarray(np.asarray(vec_ij, dtype=np.float32))
    idx_i = np.asarray(idx_i, dtype=np.int64)
    idx_m = np.asarray(idx_m, dtype=np.int64)
    n_edges = vec.shape[0]

    m = idx_m[idx_i]                                   # per-edge molecule id
    order = np.argsort(m, kind="stable")
    counts = np.bincount(m, minlength=N_MOL)
    groups = (counts + F - 1) // F                     # 512-edge groups per molecule
    gstart = np.zeros(N_MOL + 1, dtype=np.int64)
    np.cumsum(groups, out=gstart[1:])
    g_total = int(gstart[-1])

    T = max(1, -(-g_total // (P * GPT * N_CORES)))     # tiles per core
    g_padded = N_CORES * T * P * GPT
    e_padded = g_padded * F

    # destination slot of the k-th (sorted) edge of molecule m0: F*gstart[m0]+k
    starts = np.zeros(N_MOL + 1, dtype=np.int64)
    np.cumsum(counts, out=starts[1:])
    m_sorted = m[order]
    rank = np.arange(n_edges, dtype=np.int64) - starts[m_sorted]
    dst = gstart[m_sorted] * F + rank

    vp = np.empty((e_padded, 3), dtype=np.float32)
    vp[:, 0] = PAD_X
    vp[:, 1] = 0.0
    vp[:, 2] = 0.0
    vp[dst] = vec[order]

    # per-core planar tiles [T, P, 3, FH]: partition p of tile t holds edges
    # [(t*P+p)*FH, +FH) of the core's slice, x/y/z as separate planes
    in_maps = []
    per_core = T * P * FH
    for c in range(N_CORES):
        chunk = vp[c * per_core : (c + 1) * per_core]  # [T*P*FH, 3]
        vc = np.ascontiguousarray(
            chunk.reshape(T, P, FH, 3).transpose(0, 1, 3, 2).reshape(T, P, 3 * FH)
        )
        in_maps.append({"v": vc})
    return in_maps, T, gstart


def _finish(results, T, gstart):
    """Combine per-core [P, T*GPT] group sums into per-molecule totals."""
    # core c, tile t, partition p, half h -> global group ((c*T+t)*P+p)*GPT+h
    parts = [
        results[c]["gsum"].reshape(P, T, GPT).transpose(1, 0, 2).ravel()
        for c in range(N_CORES)
    ]
    gsums = np.concatenate(parts)
    csum = np.zeros(gsums.shape[0] + 1, dtype=np.float64)
    np.cumsum(gsums, dtype=np.float64, out=csum[1:])
    y = 0.5 * (csum[gstart[1:]] - csum[gstart[:-1]])
    return y.astype(np.float32)


# ---------------------------------------------------------------- entry point
def kernel(vec_ij, positions, idx_i, idx_m):
    from concourse import bass_utils

    in_maps, T, gstart = _prepare(vec_ij, idx_i, idx_m)
    nc = _build_kernel(T)
    res = bass_utils.run_bass_kernel_spmd(nc, in_maps, core_ids=list(range(N_CORES)))
    return _finish(res.results, T, gstart)


# revision 19
# speedup vs baseline: 1.0012x; 1.0012x over previous
"""LJ potential + two-level segment sum (edges -> atoms -> molecules) on 8 trn2 cores.

Strategy ("shard edges by molecule"):
  The final output is only per-molecule [1000]. On the host (as the sharding
  step) we compute each edge's molecule id m_e = idx_m[idx_i[e]], order edges
  by molecule, and pad each molecule's edge run to a multiple of 512 so every
  512-edge group (= one SBUF partition row per tile) is molecule-pure. Pad
  edges use vec=(1e3,0,0): r=1000 > cutoff => switch==0 => exactly zero energy.

  Each core gets a contiguous slice of the padded stream as [T, 128, 3*F]
  tiles (x/y/z planes, F=512; partition p of tile t holds edges
  [(t*128+p)*512, +512) of the core's slice). The device computes per-edge
  LJ energy with fused custom DVE ops + ACT + one GPSIMD add; the final
  fused DVE op carries accum=ADD, so its accum_out [128,1] directly yields
  the per-group (512-edge) sums. The [128, T] group sums are DMA'd back;
  the host slices them at (host-known) molecule group boundaries.

Per-edge math (matches reference exactly up to fp rounding):
  r2 = x^2+y^2+z^2 ;  u = 1/r2 ;  p6 = u^3 ;  q = p6^2 - p6
  a  = relu(2r - 4), r = sqrt(r2) ;  b = relu(1 - a)   # == clamp(5-2r, 0, 1)
  switch = b^2*(3-2b)                                  # == reference switch
  y  = q * switch ;  out_m = 0.5 * sum_{e in m} y_e    (0.5 applied on host)
"""

import sys

import numpy as np

if "/opt/trn_rl_repo" not in sys.path:
    sys.path.insert(0, "/opt/trn_rl_repo")

N_EDGES = 20_000_000
N_ATOMS = 1_000_000
N_MOL = 1000
N_CORES = 8
P = 128          # SBUF partitions
F = 1024         # edges per group; molecule padding grain
GPT = 1          # groups per partition row per tile
FH = F * GPT     # tile free width (edges per partition row per tile)
PAD_X = 1.0e3    # pad edge vec = (1e3, 0, 0) -> r2 = 1e6 -> switch = 0 -> y = 0

_registered_ops = {}
_compiled_cache = {}


# ---------------------------------------------------------------- custom DVE ops
def _register_custom_ops():
    """Register the fused LJ ops in concourse's custom-DVE table (idempotent)."""
    if _registered_ops:
        return _registered_ops

    from concourse import dve_ops as D
    from concourse.dve_spec import (
        AluOp, Spec, Src0, Src1, C0, One, lower, relu, sq, _has_src1,
    )
    from concourse.dve_uop import DveOpSpec

    def _shas(spec):
        out = {}
        for ver in ("v3", "v4"):
            s = DveOpSpec(
                name="tmp", opcode=1, uops=lower(spec, ver=ver), rd1_en=_has_src1(spec)
            )
            out[ver] = s.sha(ver)
        return out

    def _add(name, spec):
        existing = {op.name: op for op in D.OPS}
        if name in existing:
            _registered_ops[name] = existing[name]
            return
        op = D.DveOp(name, spec, subdim=False, uops_sha=_shas(spec))
        D.OPS.append(op)
        D._SUB_OPCODE_FOR_NAME[name] = D._CUSTOM_DVE_ROW_BASE + len(D.OPS) - 1
        assert D._SUB_OPCODE_FOR_NAME[name] < 0x20
        D.CUSTOM_DVE_SPECS[name] = spec
        _registered_ops[name] = op

    _add(
        "LJ_SQSUM2",
        Spec(
            body=sq(Src0) + sq(Src1),
            reference=lambda in0, in1, s0, s1, imm2: in0 * in0 + in1 * in1,
        ),
    )
    u3 = Src0 * Src0 * Src0
    _add(
        "LJ_Q",
        Spec(
            body=sq(u3) - u3,
            reference=lambda in0, in1, s0, s1, imm2: (
                (in0.astype(np.float32) ** 3) ** 2 - in0.astype(np.float32) ** 3
            ),
        ),
    )

    # in0=q, in1=a (=relu(2r-4)): b = relu(1-a); y = q * b^2 * (3-2b)
    # accum_out[p] = sum over the free dim of y  (per-group energy sum)
    def _ref_final2a_acc(in0, in1, s0, s1, imm2):
        b = np.maximum(1.0 - in1, 0.0).astype(np.float32)
        out = in0 * (b * b * (s0 - 2.0 * b))
        return out, out.sum(axis=-1, keepdims=True, dtype=np.float32)

    b_node = relu(One - Src1)
    _add(
        "LJ_FINAL2A_ACC",
        Spec(
            body=Src0 * (sq(b_node) * (C0 - (b_node + b_node))),
            accum=AluOp.ADD,
            reference=_ref_final2a_acc,
        ),
    )
    return _registered_ops


# ---------------------------------------------------------------- device kernel
def _build_kernel(T):
    """Build + compile the SPMD Bass program for T tiles per core."""
    if T in _compiled_cache:
        return _compiled_cache[T]

    import concourse.bacc as bacc
    import concourse.bass as bass
    import concourse.mybir as mybir
    import concourse.tile as tile
    from concourse.dve_ops import RECIPROCAL_APPROX_FAST

    ops = _register_custom_ops()
    f32 = mybir.dt.float32
    AF = mybir.ActivationFunctionType

    nc = bacc.Bacc("TRN2", target_bir_lowering=False, debug=False)
    v_dram = nc.dram_tensor("v", [T, P, 3 * FH], f32, kind="ExternalInput")
    out_dram = nc.dram_tensor("gsum", [P, T * GPT], f32, kind="ExternalOutput")

    with tile.TileContext(nc) as tc:
        with (
            tc.tile_pool(name="vin", bufs=4) as vin_pool,
            tc.tile_pool(name="work", bufs=3) as work,
            tc.tile_pool(name="cst", bufs=1) as cst,
        ):
            bias_m4 = cst.tile([P, 1], f32)
            nc.vector.memset(bias_m4[:], -4.0)
            gsum = cst.tile([P, T * GPT], f32)

            vt, x2t, yz2t, r2t = {}, {}, {}, {}

            # Software-pipelined emission: DMA 2 tiles ahead of the r2 stage,
            # which runs 2 tiles ahead of the energy/accumulate stage, so every
            # engine's program order only reaches ops whose inputs are ready.
            for ti in range(T + 4):
                if 0 <= ti - 2 < T:
                    t = ti - 2
                    v = vt.pop(t)
                    x2 = work.tile([P, FH], f32, tag="x2")
                    nc.scalar.activation(x2[:], v[:, 0:FH], AF.Square)
                    yz2 = work.tile([P, FH], f32, tag="yz2")
                    nc.vector._custom_dve(
                        ops["LJ_SQSUM2"], out=yz2[:],
                        in0=v[:, FH : 2 * FH], in1=v[:, 2 * FH : 3 * FH],
                    )
                    r2 = work.tile([P, FH], f32, tag="r2", bufs=4)
                    nc.gpsimd.tensor_add(r2[:], x2[:], yz2[:])
                    r2t[t] = r2
                if 0 <= ti - 4:
                    t = ti - 4
                    r2 = r2t.pop(t)
                    u = work.tile([P, FH], f32, tag="u")
                    nc.vector._custom_dve(RECIPROCAL_APPROX_FAST, out=u[:],
                                          in0=r2[:],
                                          s0=-0.23549792, s1=2.0017324, imm2=2.0)
                    r = work.tile([P, FH], f32, tag="r")
                    nc.scalar.activation(r[:], r2[:], AF.Sqrt)
                    a = work.tile([P, FH], f32, tag="a")
                    nc.scalar.activation(a[:], r[:], AF.Relu, bias=bias_m4[:],
                                         scale=2.0)
                    q = work.tile([P, FH], f32, tag="q")
                    nc.vector._custom_dve(ops["LJ_Q"], out=q[:], in0=u[:])

                    yv = work.tile([P, FH], f32, tag="yv")
                    nc.vector._custom_dve(
                        ops["LJ_FINAL2A_ACC"], out=yv[:], in0=q[:], in1=a[:],
                        s0=3.0, accum_out=gsum[:, t : t + 1],
                    )
                if ti < T:
                    t = ti
                    v = vin_pool.tile([P, 3 * FH], f32, tag="v")
                    nc.sync.dma_start(v[:], v_dram.ap()[t])
                    vt[t] = v

            nc.sync.dma_start(out_dram.ap()[:], gsum[:])

    nc.compile()
    _compiled_cache[T] = nc
    return nc


# ---------------------------------------------------------------- host prep
def _prepare(vec_ij, idx_i, idx_m):
    """Sort+pad edges by molecule; build per-core tile arrays and metadata.

    Returns (in_maps, T, gstart) where gstart[m] is molecule m's first
    512-edge group index in the global (concatenated-core) group order.
    """
    vec = np.ascontiguousarray(np.asarray(vec_ij, dtype=np.float32))
    idx_i = np.asarray(idx_i, dtype=np.int64)
    idx_m = np.asarray(idx_m, dtype=np.int64)
    n_edges = vec.shape[0]

    m = idx_m[idx_i]                                   # per-edge molecule id
    order = np.argsort(m, kind="stable")
    counts = np.bincount(m, minlength=N_MOL)
    groups = (counts + F - 1) // F                     # 512-edge groups per molecule
    gstart = np.zeros(N_MOL + 1, dtype=np.int64)
    np.cumsum(groups, out=gstart[1:])
    g_total = int(gstart[-1])

    T = max(1, -(-g_total // (P * GPT * N_CORES)))     # tiles per core
    g_padded = N_CORES * T * P * GPT
    e_padded = g_padded * F

    # destination slot of the k-th (sorted) edge of molecule m0: F*gstart[m0]+k
    starts = np.zeros(N_MOL + 1, dtype=np.int64)
    np.cumsum(counts, out=starts[1:])
    m_sorted = m[order]
    rank = np.arange(n_edges, dtype=np.int64) - starts[m_sorted]
    dst = gstart[m_sorted] * F + rank

    vp = np.empty((e_padded, 3), dtype=np.float32)
    vp[:, 0] = PAD_X
    vp[:, 1] = 0.0
    vp[:, 2] = 0.0
    vp[dst] = vec[order]

    # per-core planar tiles [T, P, 3, FH]: partition p of tile t holds edges
    # [(t*P+p)*FH, +FH) of the core's slice, x/y/z as separate planes
    in_maps = []
    per_core = T * P * FH
    for c in range(N_CORES):
        chunk = vp[c * per_core : (c + 1) * per_core]  # [T*P*FH, 3]
        vc = np.ascontiguousarray(
            chunk.reshape(T, P, FH, 3).transpose(0, 1, 3, 2).reshape(T, P, 3 * FH)
        )
        in_maps.append({"v": vc})
    return in_maps, T, gstart


def _finish(results, T, gstart):
    """Combine per-core [P, T*GPT] group sums into per-molecule totals."""
    # core c, tile t, partition p, half h -> global group ((c*T+t)*P+p)*GPT+h
    parts = [
        results[c]["gsum"].reshape(P, T, GPT).transpose(1, 0, 2).ravel()
        for c in range(N_CORES)
    ]
    gsums = np.concatenate(parts)
    csum = np.zeros(gsums.shape[0] + 1, dtype=np.float64)
    np.cumsum(gsums, dtype=np.float64, out=csum[1:])
    y = 0.5 * (csum[gstart[1:]] - csum[gstart[:-1]])
    return y.astype(np.float32)


# ---------------------------------------------------------------- entry point
def kernel(vec_ij, positions, idx_i, idx_m):
    from concourse import bass_utils

    in_maps, T, gstart = _prepare(vec_ij, idx_i, idx_m)
    nc = _build_kernel(T)
    res = bass_utils.run_bass_kernel_spmd(nc, in_maps, core_ids=list(range(N_CORES)))
    return _finish(res.results, T, gstart)


# revision 20
# speedup vs baseline: 1.2983x; 1.2967x over previous
"""LJ potential + two-level segment sum (edges -> atoms -> molecules) on 8 trn2 cores.

Strategy ("shard edges by molecule"):
  The final output is only per-molecule [1000]. On the host (as the sharding
  step) we compute each edge's molecule id m_e = idx_m[idx_i[e]], order edges
  by molecule, and pad each molecule's edge run to a multiple of 512 so every
  512-edge group (= one SBUF partition row per tile) is molecule-pure. Pad
  edges use vec=(1e3,0,0): r=1000 > cutoff => switch==0 => exactly zero energy.

  Each core gets a contiguous slice of the padded stream as [T, 128, 3*F]
  tiles (x/y/z planes, F=512; partition p of tile t holds edges
  [(t*128+p)*512, +512) of the core's slice). The device computes per-edge
  LJ energy with fused custom DVE ops + ACT + one GPSIMD add; the final
  fused DVE op carries accum=ADD, so its accum_out [128,1] directly yields
  the per-group (512-edge) sums. The [128, T] group sums are DMA'd back;
  the host slices them at (host-known) molecule group boundaries.

Per-edge math (matches reference exactly up to fp rounding):
  r2 = x^2+y^2+z^2 ;  u = 1/r2 ;  p6 = u^3 ;  q = p6^2 - p6
  a  = relu(2r - 4), r = sqrt(r2) ;  b = relu(1 - a)   # == clamp(5-2r, 0, 1)
  switch = b^2*(3-2b)                                  # == reference switch
  y  = q * switch ;  out_m = 0.5 * sum_{e in m} y_e    (0.5 applied on host)
"""

import sys

import numpy as np

if "/opt/trn_rl_repo" not in sys.path:
    sys.path.insert(0, "/opt/trn_rl_repo")

N_EDGES = 20_000_000
N_ATOMS = 1_000_000
N_MOL = 1000
N_CORES = 8
P = 128          # SBUF partitions
F = 1024         # edges per group; molecule padding grain
GPT = 1          # groups per partition row per tile
FH = F * GPT     # tile free width (edges per partition row per tile)
PAD_X = 1.0e3    # pad edge vec = (1e3, 0, 0) -> r2 = 1e6 -> switch = 0 -> y = 0

_registered_ops = {}
_compiled_cache = {}


# ---------------------------------------------------------------- custom DVE ops
def _register_custom_ops():
    """Register the fused LJ ops in concourse's custom-DVE table (idempotent)."""
    if _registered_ops:
        return _registered_ops

    from concourse import dve_ops as D
    from concourse.dve_spec import (
        AluOp, Spec, Src0, Src1, C0, One, lower, relu, sq, _has_src1,
    )
    from concourse.dve_uop import DveOpSpec

    def _shas(spec):
        out = {}
        for ver in ("v3", "v4"):
            s = DveOpSpec(
                name="tmp", opcode=1, uops=lower(spec, ver=ver), rd1_en=_has_src1(spec)
            )
            out[ver] = s.sha(ver)
        return out

    def _add(name, spec):
        existing = {op.name: op for op in D.OPS}
        if name in existing:
            _registered_ops[name] = existing[name]
            return
        op = D.DveOp(name, spec, subdim=False, uops_sha=_shas(spec))
        D.OPS.append(op)
        D._SUB_OPCODE_FOR_NAME[name] = D._CUSTOM_DVE_ROW_BASE + len(D.OPS) - 1
        assert D._SUB_OPCODE_FOR_NAME[name] < 0x20
        D.CUSTOM_DVE_SPECS[name] = spec
        _registered_ops[name] = op

    _add(
        "LJ_SQSUM2",
        Spec(
            body=sq(Src0) + sq(Src1),
            reference=lambda in0, in1, s0, s1, imm2: in0 * in0 + in1 * in1,
        ),
    )
    u3 = Src0 * Src0 * Src0
    _add(
        "LJ_Q",
        Spec(
            body=sq(u3) - u3,
            reference=lambda in0, in1, s0, s1, imm2: (
                (in0.astype(np.float32) ** 3) ** 2 - in0.astype(np.float32) ** 3
            ),
        ),
    )

    # in0=q, in1=a (=relu(2r-4)): b = relu(1-a); y = q * b^2 * (3-2b)
    # accum_out[p] = sum over the free dim of y  (per-group energy sum)
    def _ref_final2a_acc(in0, in1, s0, s1, imm2):
        b = np.maximum(1.0 - in1, 0.0).astype(np.float32)
        out = in0 * (b * b * (s0 - 2.0 * b))
        return out, out.sum(axis=-1, keepdims=True, dtype=np.float32)

    b_node = relu(One - Src1)
    _add(
        "LJ_FINAL2A_ACC",
        Spec(
            body=Src0 * (sq(b_node) * (C0 - (b_node + b_node))),
            accum=AluOp.ADD,
            reference=_ref_final2a_acc,
        ),
    )
    return _registered_ops


# ---------------------------------------------------------------- device kernel
def _build_kernel(T):
    """Build + compile the SPMD Bass program for T tiles per core."""
    if T in _compiled_cache:
        return _compiled_cache[T]

    import concourse.bacc as bacc
    import concourse.bass as bass
    import concourse.mybir as mybir
    import concourse.tile as tile
    from concourse.dve_ops import RECIPROCAL_APPROX_FAST

    ops = _register_custom_ops()
    f32 = mybir.dt.float32
    AF = mybir.ActivationFunctionType

    nc = bacc.Bacc("TRN2", target_bir_lowering=False, debug=False)
    v_dram = nc.dram_tensor("v", [T, P, 3 * FH], f32, kind="ExternalInput")
    out_dram = nc.dram_tensor("gsum", [P, T * GPT], f32, kind="ExternalOutput")

    with tile.TileContext(nc) as tc:
        with (
            tc.tile_pool(name="vin", bufs=4) as vin_pool,
            tc.tile_pool(name="work", bufs=3) as work,
            tc.tile_pool(name="cst", bufs=1) as cst,
        ):
            bias_m4 = cst.tile([P, 1], f32)
            nc.vector.memset(bias_m4[:], -4.0)
            gsum = cst.tile([P, T * GPT], f32)

            vt, x2t, yz2t, r2t = {}, {}, {}, {}

            # Software-pipelined emission: DMA 2 tiles ahead of the r2 stage,
            # which runs 2 tiles ahead of the energy/accumulate stage, so every
            # engine's program order only reaches ops whose inputs are ready.
            for ti in range(T + 4):
                if 0 <= ti - 2 < T:
                    t = ti - 2
                    v = vt.pop(t)
                    with tc.tile_wait_until(ti * 0.01 + 0.001):
                        x2 = work.tile([P, FH], f32, tag="x2")
                        nc.scalar.activation(x2[:], v[:, 0:FH], AF.Square)
                        yz2 = work.tile([P, FH], f32, tag="yz2")
                        nc.vector._custom_dve(
                            ops["LJ_SQSUM2"], out=yz2[:],
                            in0=v[:, FH : 2 * FH], in1=v[:, 2 * FH : 3 * FH],
                        )
                        r2 = work.tile([P, FH], f32, tag="r2", bufs=4)
                        nc.gpsimd.tensor_add(r2[:], x2[:], yz2[:])
                        r2t[t] = r2
                if 0 <= ti - 4:
                    t = ti - 4
                    r2 = r2t.pop(t)
                    with tc.tile_wait_until(ti * 0.01 + 0.002):
                        u = work.tile([P, FH], f32, tag="u")
                        nc.vector._custom_dve(RECIPROCAL_APPROX_FAST, out=u[:],
                                              in0=r2[:],
                                              s0=-0.23549792, s1=2.0017324,
                                              imm2=2.0)
                        r = work.tile([P, FH], f32, tag="r")
                        nc.scalar.activation(r[:], r2[:], AF.Sqrt)
                        a = work.tile([P, FH], f32, tag="a")
                        nc.scalar.activation(a[:], r[:], AF.Relu, bias=bias_m4[:],
                                             scale=2.0)
                        q = work.tile([P, FH], f32, tag="q")
                        nc.vector._custom_dve(ops["LJ_Q"], out=q[:], in0=u[:])

                        yv = work.tile([P, FH], f32, tag="yv")
                        nc.vector._custom_dve(
                            ops["LJ_FINAL2A_ACC"], out=yv[:], in0=q[:], in1=a[:],
                            s0=3.0, accum_out=gsum[:, t : t + 1],
                        )
                if ti < T:
                    t = ti
                    with tc.tile_wait_until(ti * 0.01):
                        v = vin_pool.tile([P, 3 * FH], f32, tag="v")
                        nc.sync.dma_start(v[:], v_dram.ap()[t])
                        vt[t] = v

            nc.sync.dma_start(out_dram.ap()[:], gsum[:])

    nc.compile()
    _compiled_cache[T] = nc
    return nc


# ---------------------------------------------------------------- host prep
def _prepare(vec_ij, idx_i, idx_m):
    """Sort+pad edges by molecule; build per-core tile arrays and metadata.

    Returns (in_maps, T, gstart) where gstart[m] is molecule m's first
    512-edge group index in the global (concatenated-core) group order.
    """
    vec = np.ascontiguousarray(np.asarray(vec_ij, dtype=np.float32))
    idx_i = np.asarray(idx_i, dtype=np.int64)
    idx_m = np.asarray(idx_m, dtype=np.int64)
    n_edges = vec.shape[0]

    m = idx_m[idx_i]                                   # per-edge molecule id
    order = np.argsort(m, kind="stable")
    counts = np.bincount(m, minlength=N_MOL)
    groups = (counts + F - 1) // F                     # 512-edge groups per molecule
    gstart = np.zeros(N_MOL + 1, dtype=np.int64)
    np.cumsum(groups, out=gstart[1:])
    g_total = int(gstart[-1])

    T = max(1, -(-g_total // (P * GPT * N_CORES)))     # tiles per core
    g_padded = N_CORES * T * P * GPT
    e_padded = g_padded * F

    # destination slot of the k-th (sorted) edge of molecule m0: F*gstart[m0]+k
    starts = np.zeros(N_MOL + 1, dtype=np.int64)
    np.cumsum(counts, out=starts[1:])
    m_sorted = m[order]
    rank = np.arange(n_edges, dtype=np.int64) - starts[m_sorted]
    dst = gstart[m_sorted] * F + rank

    vp = np.empty((e_padded, 3), dtype=np.float32)
    vp[:, 0] = PAD_X
    vp[:, 1] = 0.0
    vp[:, 2] = 0.0
    vp[dst] = vec[order]

    # per-core planar tiles [T, P, 3, FH]: partition p of tile t holds edges
    # [(t*P+p)*FH, +FH) of the core's slice, x/y/z as separate planes
    in_maps = []
    per_core = T * P * FH
    for c in range(N_CORES):
        chunk = vp[c * per_core : (c + 1) * per_core]  # [T*P*FH, 3]
        vc = np.ascontiguousarray(
            chunk.reshape(T, P, FH, 3).transpose(0, 1, 3, 2).reshape(T, P, 3 * FH)
        )
        in_maps.append({"v": vc})
    return in_maps, T, gstart


def _finish(results, T, gstart):
    """Combine per-core [P, T*GPT] group sums into per-molecule totals."""
    # core c, tile t, partition p, half h -> global group ((c*T+t)*P+p)*GPT+h
    parts = [
        results[c]["gsum"].reshape(P, T, GPT).transpose(1, 0, 2).ravel()
        for c in range(N_CORES)
    ]
    gsums = np.concatenate(parts)
    csum = np.zeros(gsums.shape[0] + 1, dtype=np.float64)
    np.cumsum(gsums, dtype=np.float64, out=csum[1:])
    y = 0.5 * (csum[gstart[1:]] - csum[gstart[:-1]])
    return y.astype(np.float32)


# ---------------------------------------------------------------- entry point
def kernel(vec_ij, positions, idx_i, idx_m):
    from concourse import bass_utils

    in_maps, T, gstart = _prepare(vec_ij, idx_i, idx_m)
    nc = _build_kernel(T)
    res = bass_utils.run_bass_kernel_spmd(nc, in_maps, core_ids=list(range(N_CORES)))
    return _finish(res.results, T, gstart)
